# revision 108
# baseline (speedup 1.0000x reference)
"""Trainium2 Bass kernel for nn_BarrierPolicy (CBF-QP safety filter), v2.

Data-parallel over batch: 8 cores x 32768 samples, all math bf16 on-chip.
Host pre-casts x to bf16; in/out DRAM layouts are fully contiguous
(partition P, col 8c+j <-> sample 256P+c), so each x/u slab DMA is one
1KB-run descriptor per partition.

Phase A (per 4-tile super-block of 8192 samples): PE-transposes x to SP2
(partition 8m+j: 16 samples' coords per column), runs the 3-layer MLP +
dynamics matmuls with 512-wide moving dim, relu+bias evacuation split
ACT/DVE (GPSIMD cannot touch PSUM), then transposes px/g/(-2Ax) back to
xview in batched single-bank PSUM groups.

Phase B (per super-block chunk, pipelined behind phase A): Newton-form
Kiwiel variable-fixing for the box-QP dual:
  lam' = clip(lam - c(lam)/den, 0, LAMCAP),  den = sum of q over the
  not-yet-fixed set; coords are fixed one-sided when uhat == bvs,
  bvs = +-1 from sign(-c), implemented as uhat != bvs (not_equal mask).
Init lam is the all-free Newton step (num-form). Per-sample j=8 reductions
are 2-level bf16 tree-adds + f32 final (TensorReduce is always 1x mode);
per-sample scalars broadcast through a duplicated-pair (128,CL,2) view that
keeps DVE TT in 2x mode; clip/is_ge/bitmask ops ride 4x tensor_scalar. The
whole iteration chain stays on DVE (cross-engine hops cost more than they
save); T_NEWTON=3 + closed-form final u = clip(lam*g - p) gives rel err
~1.7e-2 vs the exact-bisection reference (budget 2e-2).
"""
import numpy as np

B_FULL, N = 262144, 8
NCORES = 8
S = B_FULL // NCORES          # 32768 samples per core
TILE = 2048
NT = S // TILE                # 16 tiles
SUP = 4                       # tiles per super-block
NSUP = NT // SUP
MV = 128 * SUP                # matmul moving width per super (512)
FC = S // 16                  # 2048 xview cols per core
NSLOT = S // 128              # 256 slot cols per core
NCH = 4                       # phase-B chunks
CF = FC // NCH                # 1024
CL = NSLOT // NCH             # 128
SPC = NSUP // NCH             # supers per chunk
T_NEWTON = 3
LAMCAP = float(2.0 ** 40)
EPS = 1e-9

_CACHE = {}

_CSHAPES_BF = dict(TL2=(128, 128), TL3S=(128, 32), TDA=(128, 128),
                   TDG=(128, 128), ID128H=(128, 128), B31F=(128, 8),
                   **{f"TL1E{b}": (128, 128) for b in range(8)})
_CSHAPES_F32 = dict(B1v=(128, 1), B2v=(128, 1), B32s=(128, 1))


def _consts(W1, b1, W21, b21, W22, b22, W31, b31, W32, b32, A, G):
    import ml_dtypes
    f32 = np.float32
    bf = ml_dtypes.bfloat16
    out = {}
    for b in range(8):
        T = np.zeros((128, 128), f32)
        for s0 in range(2):
            T[16 * b + 8 * s0:16 * b + 8 * s0 + 8, 64 * s0:64 * s0 + 64] = W1
        out[f"TL1E{b}"] = T.astype(bf)
    TL2 = np.zeros((128, 128), f32)
    for s0 in range(2):
        TL2[64 * s0:64 * s0 + 64, 32 * s0:32 * s0 + 32] = W21
        TL2[64 * s0:64 * s0 + 64, 64 + 32 * s0:64 + 32 * s0 + 32] = W22
    # stacked L3: out col m = 16*s0 + mm, mm in 0..7 -> px_j, mm=8 -> alpha raw
    TL3S = np.zeros((128, 32), f32)
    for s0 in range(2):
        TL3S[32 * s0:32 * s0 + 32, 16 * s0:16 * s0 + 8] = W31
        TL3S[64 + 32 * s0:64 + 32 * s0 + 32, 16 * s0 + 8:16 * s0 + 9] = W32
    TDA = np.kron(np.eye(16, dtype=f32), (-2.0 * A.T).astype(f32))  # -2 A x
    TDG = np.kron(np.eye(16, dtype=f32), (-2.0 * G).astype(f32))    # -2 G^T x
    out.update(TL2=TL2.astype(bf), TL3S=TL3S.astype(bf),
               TDA=TDA.astype(bf), TDG=TDG.astype(bf),
               ID128H=np.eye(128, dtype=f32).astype(bf))
    out["B1v"] = np.concatenate([b1, b1]).reshape(128, 1).astype(f32)
    out["B2v"] = np.concatenate([b21, b21, b22, b22]).reshape(128, 1).astype(f32)
    out["B31F"] = np.tile(b31.astype(f32), (128, 1)).astype(bf)
    out["B32s"] = np.full((128, 1), float(b32[0]), f32)
    return out


def build_kernel(nc, tc, x_d, u_d, cds):
    from concourse import mybir
    f32 = mybir.dt.float32
    bf16 = mybir.dt.bfloat16
    AL = mybir.AluOpType
    AF = mybir.ActivationFunctionType
    V, GP, SC = nc.vector, nc.gpsimd, nc.scalar

    with (
        tc.tile_pool(name="const", bufs=1) as cpool,
        tc.tile_pool(name="pers", bufs=1) as pers,
        tc.tile_pool(name="work", bufs=3) as work,
        tc.tile_pool(name="psT", bufs=2, space="PSUM") as psT,
        tc.tile_pool(name="psM", bufs=2, space="PSUM") as psM,
        tc.tile_pool(name="psX", bufs=1, space="PSUM") as psX,
    ):
        C = {}
        for k, v in _CSHAPES_BF.items():
            C[k] = cpool.tile(list(v), bf16, tag=k, name=k)
        for k, v in _CSHAPES_F32.items():
            C[k] = cpool.tile(list(v), f32, tag=k, name=k)
        early = ["ID128H"] + [f"TL1E{b}" for b in range(8)]
        for k in early:
            nc.sync.dma_start(C[k][:], cds[k][:])
        for k in list(_CSHAPES_BF) + list(_CSHAPES_F32):
            if k not in early:
                nc.gpsimd.dma_start(C[k][:], cds[k][:])

        def fc_bf(tag):
            return pers.tile([128, FC], bf16, tag=tag, name=tag)

        def sl_f32(tag):
            return pers.tile([128, NSLOT], f32, tag=tag, name=tag)

        # persistent full-width tensors (xview layout)
        xvb = fc_bf("xvb")
        g_xv, px_xv = fc_bf("g_xv"), fc_bf("px_xv")
        gt, pt, qq = fc_bf("gt"), fc_bf("pt"), fc_bf("qq")
        sgn = fc_bf("sgn")
        ur, uh, rb = fc_bf("ur"), fc_bf("uh"), fc_bf("rb")
        nf = fc_bf("nf")
        prodA, sqx = fc_bf("prodA"), fc_bf("sqx")
        uout = fc_bf("uout")
        l1a = pers.tile([128, FC // 2], bf16, tag="l1a", name="l1a")
        l2a = pers.tile([128, FC // 4], bf16, tag="l2a", name="l2a")
        l1b = pers.tile([128, FC // 2], bf16, tag="l1b", name="l1b")
        l2b = pers.tile([128, FC // 4], bf16, tag="l2b", name="l2b")
        # per-sample slots (f32) and dup-pair broadcasts (bf16)
        c0s, lfh, sxx = sl_f32("c0s"), sl_f32("lfh"), sl_f32("sxx")
        csum, dsum, cc = sl_f32("csum"), sl_f32("dsum"), sl_f32("cc")
        st = sl_f32("st")
        araw = pers.tile([128, NSLOT], bf16, tag="araw", name="araw")
        al4 = pers.tile([128, NSLOT], bf16, tag="al4", name="al4")
        lam2 = pers.tile([128, NSLOT, 2], bf16, tag="lam2", name="lam2")
        bvs2 = pers.tile([128, NSLOT, 2], bf16, tag="bvs2", name="bvs2")

        x8 = lambda ap: ap.rearrange("p (c j) -> p c j", j=8)
        x4 = lambda ap: ap.rearrange("p (c j) -> p c j", j=4)
        x2v = lambda ap: ap.rearrange("p (c j) -> p c j", j=2)

        def tree8(src, out_f32, sl, eng_last=GP, eng_lvl=None, eps=None):
            """out[:, sl] = sum over j=8 of src[:, 8*sl]: 2 bf16 levels + f32."""
            eng_lvl = eng_lvl or V
            fs = slice(sl.start * 8, sl.stop * 8)
            h1s = slice(sl.start * 4, sl.stop * 4)
            h2s = slice(sl.start * 2, sl.stop * 2)
            la, lb = (l1a, l2a) if src is not qq else (l1b, l2b)
            s8 = x8(src[:, fs])
            eng_lvl.tensor_tensor(x4(la[:, h1s]), s8[:, :, 0:4], s8[:, :, 4:8],
                                  AL.add)
            eng_lvl.tensor_tensor(x2v(lb[:, h2s]), x4(la[:, h1s])[:, :, 0:2],
                                  x4(la[:, h1s])[:, :, 2:4], AL.add)
            if eps is None:
                eng_last.tensor_tensor(out_f32[:, sl], x2v(lb[:, h2s])[:, :, 0],
                                       x2v(lb[:, h2s])[:, :, 1], AL.add)
            else:
                eng_last.scalar_tensor_tensor(
                    out_f32[:, sl], x2v(lb[:, h2s])[:, :, 0], eps,
                    x2v(lb[:, h2s])[:, :, 1], AL.add, AL.add)

        # ---------------- Phase A (per super-block) ----------------
        # contiguous layout: partition P, col 8c+j <-> sample 256P + c, coord j
        x_flat = x_d.rearrange("(P c) j -> P (c j)", P=128)
        u_flat = u_d.rearrange("(P c) j -> P (c j)", P=128)

        def phase_a(sp):
            cs = slice(MV * sp, MV * sp + MV)               # xview cols
            ss = slice(16 * SUP * sp, 16 * SUP * (sp + 1))  # slot cols
            nc.sync.dma_start(xvb[:, cs], x_flat[:, cs])
            TPx = psT.tile([128, SUP, 128], bf16, tag="TPx", name="TPx")
            for t in range(SUP):
                tt = SUP * sp + t
                nc.tensor.transpose(TPx[:, t, :], xvb[:, 128 * tt:128 * tt + 128],
                                    C["ID128H"][:])
            xsp2 = work.tile([128, MV], bf16, tag="xsp2", name="xsp2")
            if sp == 0:
                V.tensor_copy(xsp2[:].rearrange("p (t c) -> p t c", t=SUP),
                              TPx[:])
            else:
                SC.activation(xsp2[:].rearrange("p (t c) -> p t c", t=SUP),
                              TPx[:], AF.Copy)

            # dynamics: -2Ax and g = -2G^T x in SP2, then transpose to xview
            dyP = psM.tile([128, 2, MV], f32, tag="mmP", name="dyP")
            nc.tensor.matmul(dyP[:, 0, :], C["TDA"][:], xsp2[:])
            nc.tensor.matmul(dyP[:, 1, :], C["TDG"][:], xsp2[:])
            dyS = work.tile([128, 2, MV], bf16, tag="dyS", name="dyS")
            SC.activation(dyS[:], dyP[:], AF.Copy)

            trP = psX.tile([128, 2, SUP, 128], bf16, tag="trP", name="trP")
            for t in range(SUP):
                nc.tensor.transpose(trP[:, 0, t, :],
                                    dyS[:, 0, 128 * t:128 * t + 128],
                                    C["ID128H"][:])
                nc.tensor.transpose(trP[:, 1, t, :],
                                    dyS[:, 1, 128 * t:128 * t + 128],
                                    C["ID128H"][:])
            # prodA = (-2Ax)_xv * x_xv ; g_xv evac ; sqx = x*x
            V.tensor_tensor(prodA[:, cs].rearrange("p (t c) -> p t c", t=SUP),
                            trP[:, 0, :, :],
                            xvb[:, cs].rearrange("p (t c) -> p t c", t=SUP),
                            AL.mult)
            if sp <= 1:
                V.tensor_copy(g_xv[:, cs].rearrange("p (t c) -> p t c", t=SUP),
                              trP[:, 1, :, :])
            else:
                SC.activation(g_xv[:, cs].rearrange("p (t c) -> p t c", t=SUP),
                              trP[:, 1, :, :], AF.Copy)
            GP.tensor_tensor(sqx[:, cs], xvb[:, cs], xvb[:, cs], AL.mult)

            h1 = work.tile([128, 8, MV], bf16, tag="h1", name="h1")
            x2 = work.tile([128, 8, MV], bf16, tag="x2", name="x2")
            if sp <= 1:
                ev1 = [SC, V, SC, V]
                ev2 = [SC, V, SC, V]
            else:
                ev1 = [SC, SC, SC, SC]
                ev2 = [SC, SC, SC, SC]
            for qr in range(4):
                mmP = psM.tile([128, 2, MV], f32, tag="mmP", name="mmP")
                for bi in range(2):
                    nc.tensor.matmul(mmP[:, bi, :], C[f"TL1E{2 * qr + bi}"][:],
                                     xsp2[:])
                hs = slice(2 * qr, 2 * qr + 2)
                e = ev1[qr]
                if e is SC:
                    SC.activation(h1[:, hs, :], mmP[:], AF.Relu, bias=C["B1v"][:])
                else:
                    e.tensor_scalar(h1[:, hs, :], mmP[:], C["B1v"][:], 0.0,
                                    AL.add, AL.max)
            for qr in range(4):
                mmP = psM.tile([128, 2, MV], f32, tag="mmP", name="mmP")
                for bi in range(2):
                    nc.tensor.matmul(mmP[:, bi, :], C["TL2"][:],
                                     h1[:, 2 * qr + bi, :])
                hs = slice(2 * qr, 2 * qr + 2)
                e = ev2[qr]
                if e is SC:
                    SC.activation(x2[:, hs, :], mmP[:], AF.Relu, bias=C["B2v"][:])
                else:
                    e.tensor_scalar(x2[:, hs, :], mmP[:], C["B2v"][:], 0.0,
                                    AL.add, AL.max)

            # L3: b = 2g+k -> out partitions 32g+16s0+mm, psum slot k
            LA = psM.tile([128, 2, MV], f32, tag="mmP", name="LA")
            for b in range(8):
                g4, k2 = b // 2, b % 2
                nc.tensor.matmul(LA[32 * g4:32 * g4 + 32, k2, :], C["TL3S"][:],
                                 x2[:, b, :], tile_position=(0, 32 * g4))
            pxal = work.tile([128, 2, MV], bf16, tag="pxal", name="pxal")
            if sp == 0:
                V.tensor_copy(pxal[:], LA[:])
            else:
                SC.activation(pxal[:], LA[:], AF.Copy)

            # px/alpha transpose back: pxal (128, 2, MV) -> 2*SUP blocks
            paT = psX.tile([128, 2 * SUP, 128], bf16, tag="paT", name="paT")
            for k in range(2):
                for t in range(SUP):
                    nc.tensor.transpose(paT[:, SUP * k + t, :],
                                        pxal[:, k, 128 * t:128 * t + 128],
                                        C["ID128H"][:])
            # px_xv[r, 128t+32g+16k+8s0+j] = paT[r, SUP*k+t, 32g+16s0+j] + b31[j]
            pxv6 = px_xv[:, cs].rearrange("p (t g k s j) -> p t k g s j",
                                          t=SUP, k=2, g=4, s=2, j=8)
            pat6 = paT.rearrange("p (k t) (g s m) -> p t k g s m",
                                 k=2, g=4, s=2, m=16)
            arw5 = araw[:, ss].rearrange("p (t g k s) -> p t g k s",
                                         t=SUP, g=4, k=2, s=2)
            for k in range(2):
                for s in range(2):
                    V.tensor_tensor(
                        pxv6[:, :, k, :, s, :], pat6[:, :, k, :, s, 0:8],
                        C["B31F"][:, None, None, :].broadcast_to(
                            (128, SUP, 4, 8)),
                        AL.add)
                SC.activation(arw5[:, :, :, k, :], pat6[:, :, k, :, :, 8],
                              AF.Copy)

        # ---------------- Phase B setup (per chunk) ----------------
        def setup_chunk(ch):
            fs = slice(CF * ch, CF * ch + CF)
            sl = slice(CL * ch, CL * ch + CL)
            # c0 = lfh + 4*sigmoid(araw + b32)*(16 - sxx)
            tree8(prodA, lfh, sl, GP, GP)
            tree8(sqx, sxx, sl, GP, GP)
            SC.activation(al4[:, sl], araw[:, sl], AF.Sigmoid, bias=C["B32s"][:])
            GP.tensor_scalar(st[:, sl], sxx[:, sl], -4.0, 64.0, AL.mult, AL.add)
            GP.tensor_tensor(c0s[:, sl], al4[:, sl], st[:, sl], AL.mult)
            GP.tensor_tensor(c0s[:, sl], c0s[:, sl], lfh[:, sl], AL.add)
            # transform: gt = |g|, pt = sign(g)*p, q = gt^2
            SC.activation(gt[:, fs], g_xv[:, fs], AF.Abs)
            SC.activation(sgn[:, fs], g_xv[:, fs], AF.Sign)
            V.tensor_tensor(pt[:, fs], sgn[:, fs], px_xv[:, fs], AL.mult)
            V.tensor_tensor(qq[:, fs], gt[:, fs], gt[:, fs], AL.mult)
            # init lam = clip(-(c0 - sum gt*pt)/(sum q + eps), 0, LAMCAP)
            V.tensor_tensor(rb[:, fs], gt[:, fs], pt[:, fs], AL.mult)
            tree8(rb, csum, sl, V)
            tree8(qq, dsum, sl, V, eps=EPS)
            V.tensor_tensor(cc[:, sl], c0s[:, sl], csum[:, sl], AL.subtract)
            V.reciprocal(dsum[:, sl], dsum[:, sl])
            V.tensor_tensor(st[:, sl], cc[:, sl], dsum[:, sl], AL.mult)
            V.tensor_scalar(st[:, sl], st[:, sl], -1.0, 0.0, AL.mult, AL.max)
            V.tensor_scalar(lam2[:, sl, :],
                            st[:, sl, None].broadcast_to((128, CL, 2)),
                            LAMCAP, None, AL.min)

        def l2v(ap_pair, sl):
            # dup-pair bf16 slot view broadcast to (128, n, 4, 2)
            return ap_pair[:, sl, None, :].broadcast_to(
                (128, sl.stop - sl.start, 4, 2))

        def xpair(ap, sl):
            fs = slice(sl.start * 8, sl.stop * 8)
            return ap[:, fs].rearrange("p (c k two) -> p c k two", k=4, two=2)

        def iter_chunk(ch, W=1, half=None):
            CLW, CFW = CL * W, CF * W
            sl = slice(CLW * ch, CLW * ch + CLW)
            fs = slice(CFW * ch, CFW * ch + CFW)
            if half is not None:
                CLW, CFW = CLW // 2, CFW // 2
                sl = slice(sl.start + CLW * half, sl.start + CLW * (half + 1))
                fs = slice(fs.start + CFW * half, fs.start + CFW * (half + 1))
            V.tensor_tensor(xpair(ur, sl), l2v(lam2, sl), xpair(gt, sl),
                            AL.mult)
            V.tensor_tensor(ur[:, fs], ur[:, fs], pt[:, fs], AL.subtract)
            V.tensor_scalar(uh[:, fs], ur[:, fs], 1.0, -1.0, AL.min, AL.max)
            V.tensor_tensor(rb[:, fs], gt[:, fs], uh[:, fs], AL.mult)
            tree8(rb, csum, sl, V)
            V.tensor_tensor(cc[:, sl], c0s[:, sl], csum[:, sl], AL.add)
            # bvs: +1 if c < 0 else -1 (c==0 -> -1, fixes low)
            V.tensor_scalar(st[:, sl], cc[:, sl], 0.0, None, AL.is_ge)
            V.tensor_scalar(bvs2[:, sl, :],
                            st[:, sl, None].broadcast_to((128, CLW, 2)),
                            -2.0, 1.0, AL.mult, AL.add)
            V.tensor_tensor(xpair(nf, sl), xpair(uh, sl), l2v(bvs2, sl),
                            AL.not_equal)
            V.tensor_tensor(qq[:, fs], qq[:, fs], nf[:, fs], AL.mult)
            tree8(qq, dsum, sl, V, eps=EPS)
            V.reciprocal(dsum[:, sl], dsum[:, sl])
            GP.tensor_tensor(st[:, sl], cc[:, sl], dsum[:, sl], AL.mult)
            V.tensor_tensor(lam2[:, sl, :], lam2[:, sl, :],
                            st[:, sl, None].broadcast_to((128, CLW, 2)),
                            AL.subtract)
            GP.tensor_scalar(lam2[:, sl, :], lam2[:, sl, :], 0.0, LAMCAP,
                             AL.max, AL.min)

        def final_chunk(ch):
            sl = slice(CL * ch, CL * ch + CL)
            fs = slice(CF * ch, CF * ch + CF)
            V.tensor_tensor(xpair(ur, sl), l2v(lam2, sl), xpair(g_xv, sl),
                            AL.mult)
            V.tensor_tensor(ur[:, fs], ur[:, fs], px_xv[:, fs], AL.subtract)
            V.tensor_scalar(uout[:, fs], ur[:, fs], 1.0, -1.0, AL.min, AL.max)
            nc.sync.dma_start(u_flat[:, fs], uout[:, fs])

        # ---------------- emission order (pipelined) ----------------
        for ch in range(NCH):
            for s in range(SPC):
                phase_a(SPC * ch + s)
            setup_chunk(ch)
        for it in range(T_NEWTON):
            for ch in range(NCH):
                iter_chunk(ch)
        for ch in range(NCH):
            final_chunk(ch)


def _build():
    from concourse import bacc, mybir
    from concourse import tile as tile_mod
    from concourse._compat import axon_active
    bf16 = mybir.dt.bfloat16
    nc = bacc.Bacc("TRN2", target_bir_lowering=False,
                   debug=not axon_active(), num_devices=NCORES)
    x_d = nc.dram_tensor("x", [S, N], bf16, kind="ExternalInput").ap()
    u_d = nc.dram_tensor("u", [S, N], bf16, kind="ExternalOutput").ap()
    cds = {}
    for k, v in _CSHAPES_BF.items():
        cds[k] = nc.dram_tensor(k, list(v), bf16, kind="ExternalInput").ap()
    for k, v in _CSHAPES_F32.items():
        cds[k] = nc.dram_tensor(k, list(v), mybir.dt.float32,
                                kind="ExternalInput").ap()
    with tile_mod.TileContext(nc) as tc:
        build_kernel(nc, tc, x_d, u_d, cds)
    nc.compile()
    return nc


def kernel(x, W1, b1, W21, b21, W22, b22, W31, b31, W32, b32, A, G, mean, std):
    import ml_dtypes
    from concourse.bass_utils import run_bass_kernel_spmd
    f32 = np.float32
    bf = ml_dtypes.bfloat16
    x = np.asarray(x, f32)
    x0 = (x * np.asarray(std, f32) + np.asarray(mean, f32)).astype(bf)

    consts = _consts(np.asarray(W1, f32), np.asarray(b1, f32), np.asarray(W21, f32),
                     np.asarray(b21, f32), np.asarray(W22, f32), np.asarray(b22, f32),
                     np.asarray(W31, f32), np.asarray(b31, f32), np.asarray(W32, f32),
                     np.asarray(b32, f32), np.asarray(A, f32), np.asarray(G, f32))
    if "nc" not in _CACHE:
        _CACHE["nc"] = _build()
    nc = _CACHE["nc"]

    in_maps = []
    for c in range(NCORES):
        m = {"x": np.ascontiguousarray(x0[c * S:(c + 1) * S])}
        m.update(consts)
        in_maps.append(m)
    res = run_bass_kernel_spmd(nc, in_maps, list(range(NCORES)))
    out = np.concatenate([np.asarray(res.results[c]["u"]).astype(f32)
                          for c in range(NCORES)], axis=0)
    return out
